# revision 15
# baseline (speedup 1.0000x reference)
"""FFM layer (embedding lookup + field-factorization) on 8 trn2 NeuronCores.

Data-parallel over batch (4096 rows -> 512/core), reduced table replicated.
The reference sums v[idx] over the per-feature field axis before any use, so
each feature contributes only vsum[t,:] = sum_f v[t,f,:] (K=8 floats), and
the sq term needs only vsq[t] = sum_k vsum[t,k]^2.  Host precomputes the
index-independent 9-float row (padded to the 256 B SWDGE minimum):
    row[t] = [vsum[t,0..7] | w[t] + w0/26 - 0.5*vsq[t]]
so per batch row:  acc[b,:] = sum_j row[gidx[b,j]];
                   out[b]   = acc[8] + 0.5*sum_k acc[k]^2.
Lookups: 26 SWDGE dma_gathers (one per field, 512 int16 idx each) issued
round-robin over 4 SWDGE queues -- the 4 per-queue Q7 generation/drain
contexts overlap, turning the serial ~4.4 us/gather chain into ~4 gathers
per ~4.7 us (random-read-latency/gen-fixed-cost balanced).  num_idxs lives
in one pre-loaded register (no per-gather MOVE).  VectorE reduces the field
axis in four partial groups (the first three overlap later gathers) and runs
the tiny quadratic tail; vsum is pre-scaled by 1/sqrt(2) so the 0.5 factor
needs no extra multiply.
"""

import sys

import numpy as np

FIELD = 26
K = 8
RPAD = 64                # padded row length in f32 (256 B)
VOCAB = 20000
TOTAL = FIELD * VOCAB    # 520000
B = 4096
NCORES = 8
BC = B // NCORES         # 512 batch rows per core
P = 128
NTILES = BC // P         # 4
NSLOT = BC // 16         # 32 int16 index slots per idx partition

_TRN_REPO = "/opt/trn_rl_repo"

_cache = {}


def _build_nc(n_iters=1):
    if _TRN_REPO not in sys.path:
        sys.path.insert(0, _TRN_REPO)
    from concourse import bacc, mybir, tile

    f32 = mybir.dt.float32
    i16 = mybir.dt.int16
    Alu = mybir.AluOpType
    Ax = mybir.AxisListType

    nc = bacc.Bacc("TRN2", target_bir_lowering=False, debug=False,
                   num_swdge_queues=4, enable_partition_id=False,
                   monotonic_sem_count=0)
    idx_d = nc.dram_tensor("idx16", [P, FIELD, NSLOT], i16,
                           kind="ExternalInput")
    taug_d = nc.dram_tensor("taug", [TOTAL, RPAD], f32, kind="ExternalInput")
    out_d = nc.dram_tensor("out", [BC, 1], f32, kind="ExternalOutput")

    with tile.TileContext(nc) as tc:
        with tc.tile_pool(name="main", bufs=1) as pool:
            for _ in range(n_iters):
                idx_sb = pool.tile([P, FIELD, NSLOT], i16, tag="idx")
                nc.sync.dma_start(out=idx_sb[:], in_=idx_d[:, :, :])

                # vg[p, f, t, :] = row[idx[t*128+p, f]]
                vg = pool.tile([P, FIELD, NTILES, RPAD], f32, tag="vg")
                nreg = nc.gpsimd.alloc_register()
                nc.gpsimd.reg_mov(nreg, BC)
                for f in range(FIELD):
                    nc.gpsimd.dma_gather(
                        out_ap=vg[:, f],
                        in_ap=taug_d[f * VOCAB:(f + 1) * VOCAB, :],
                        idxs_ap=idx_sb[:, f, :],
                        num_idxs=BC,
                        num_idxs_reg=nreg,
                        elem_size=RPAD,
                        single_packet=False,
                        queue_num=f % 4,
                    )

                # four partial reduces; the first three overlap later gathers
                GRP = [(0, 7), (7, 14), (14, 20), (20, 26)]
                acc4 = pool.tile([P, NTILES, 9, 4], f32, tag="acc4")
                for gi, (f0, f1) in enumerate(GRP):
                    nc.vector.tensor_reduce(
                        out=acc4[:, :, :, gi],
                        in_=vg[:, f0:f1, :, :9].rearrange(
                            "p f t c -> p t c f"
                        ),
                        axis=Ax.X,
                        op=Alu.add,
                    )
                acc = pool.tile([P, NTILES, 9], f32, tag="acc")
                nc.vector.tensor_reduce(
                    out=acc[:], in_=acc4[:], axis=Ax.X, op=Alu.add
                )
                # vsum was scaled by 1/sqrt(2) host-side, so
                # out = acc[8] + sum_k acc[k]^2 directly
                ssq = pool.tile([P, NTILES, K], f32, tag="ssq")
                nc.vector.tensor_tensor(
                    out=ssq[:], in0=acc[:, :, :K], in1=acc[:, :, :K],
                    op=Alu.mult,
                )
                s2 = pool.tile([P, NTILES], f32, tag="s2")
                nc.vector.tensor_reduce(
                    out=s2[:], in_=ssq[:], axis=Ax.X, op=Alu.add
                )
                out_all = pool.tile([P, NTILES], f32, tag="oa")
                nc.vector.tensor_tensor(
                    out=out_all[:], in0=s2[:], in1=acc[:, :, K],
                    op=Alu.add,
                )
                nc.sync.dma_start(
                    out=out_d[:, :].rearrange("(t p) one -> p (t one)", p=P),
                    in_=out_all[:],
                )
    nc.compile()
    return nc


def get_nc():
    if "nc" not in _cache:
        _cache["nc"] = _build_nc()
    return _cache["nc"]


def make_in_maps(inputs, offsets, w0, w, v):
    del offsets
    inp = np.asarray(inputs)
    idx16 = np.ascontiguousarray(
        inp.astype(np.int16).reshape(NCORES, BC, FIELD)
    )
    v_ = np.asarray(v, dtype=np.float32).reshape(TOTAL, FIELD, K)
    vsum = v_.sum(axis=1, dtype=np.float64)
    vsq = (vsum * vsum).sum(axis=1)
    w0_ = float(np.asarray(w0, np.float64).reshape(()))
    c = (np.asarray(w, np.float64).reshape(TOTAL) + w0_ / FIELD - 0.5 * vsq)
    taug = np.zeros((TOTAL, RPAD), dtype=np.float32)
    # scale vsum by 1/sqrt(2): sum_k (acc'_k)^2 == 0.5 * sum_k acc_k^2,
    # folding the 0.5 into the table and dropping a vector op on-device
    taug[:, :K] = (vsum / np.sqrt(2.0)).astype(np.float32)
    taug[:, K] = c.astype(np.float32)
    maps = []
    for i in range(NCORES):
        shard = idx16[i]                       # [BC, FIELD]
        wrapped = shard.reshape(NSLOT, 16, FIELD).transpose(1, 2, 0)
        rep = np.ascontiguousarray(np.tile(wrapped, (P // 16, 1, 1)))
        maps.append({"idx16": rep, "taug": taug})
    return maps


def kernel(inputs, offsets, w0, w, v):
    if _TRN_REPO not in sys.path:
        sys.path.insert(0, _TRN_REPO)
    from concourse.bass_utils import run_bass_kernel_spmd

    nc = get_nc()
    in_maps = make_in_maps(inputs, offsets, w0, w, v)
    res = run_bass_kernel_spmd(nc, in_maps, list(range(NCORES)))
    out = np.concatenate(
        [np.asarray(res.results[i]["out"]) for i in range(NCORES)], axis=0
    )
    return out.astype(np.float32)


# revision 16
# speedup vs baseline: 1.0295x; 1.0295x over previous
"""FFM layer (embedding lookup + field-factorization) on 8 trn2 NeuronCores.

Data-parallel over batch (4096 rows -> 512/core), reduced table replicated.
The reference sums v[idx] over the per-feature field axis before any use, so
each feature contributes only vsum[t,:] = sum_f v[t,f,:] (K=8 floats), and
the sq term needs only vsq[t] = sum_k vsum[t,k]^2.  Host precomputes the
index-independent 9-float row (padded to the 256 B SWDGE minimum):
    row[t] = [vsum[t,0..7] | w[t] + w0/26 - 0.5*vsq[t]]
so per batch row:  acc[b,:] = sum_j row[gidx[b,j]];
                   out[b]   = acc[8] + 0.5*sum_k acc[k]^2.
Lookups: 26 SWDGE dma_gathers (one per field, 512 int16 idx each) issued
round-robin over 4 SWDGE queues -- the 4 per-queue Q7 generation/drain
contexts overlap, turning the serial ~4.4 us/gather chain into ~4 gathers
per ~4.7 us (random-read-latency/gen-fixed-cost balanced).  num_idxs lives
in one pre-loaded register (no per-gather MOVE).  VectorE reduces the field
axis in four partial groups (the first three overlap later gathers) and runs
the tiny quadratic tail; vsum is pre-scaled by 1/sqrt(2) so the 0.5 factor
needs no extra multiply.
"""

import sys

import numpy as np

FIELD = 26
K = 8
RPAD = 64                # padded row length in f32 (256 B)
VOCAB = 20000
TOTAL = FIELD * VOCAB    # 520000
B = 4096
NCORES = 8
BC = B // NCORES         # 512 batch rows per core
P = 128
NTILES = BC // P         # 4
NSLOT = BC // 16         # 32 int16 index slots per idx partition

_TRN_REPO = "/opt/trn_rl_repo"

_cache = {}


def _build_nc(n_iters=1):
    if _TRN_REPO not in sys.path:
        sys.path.insert(0, _TRN_REPO)
    from concourse import bacc, mybir, tile

    f32 = mybir.dt.float32
    i16 = mybir.dt.int16
    Alu = mybir.AluOpType
    Ax = mybir.AxisListType

    nc = bacc.Bacc("TRN2", target_bir_lowering=False, debug=False,
                   num_swdge_queues=4, enable_partition_id=False,
                   monotonic_sem_count=0)
    idx_d = nc.dram_tensor("idx16", [P, FIELD, NSLOT], i16,
                           kind="ExternalInput")
    taug_d = nc.dram_tensor("taug", [TOTAL, RPAD], f32, kind="ExternalInput")
    out_d = nc.dram_tensor("out", [BC, 1], f32, kind="ExternalOutput")

    with tile.TileContext(nc) as tc:
        with tc.tile_pool(name="main", bufs=1) as pool:
            for _ in range(n_iters):
                idx_sb = pool.tile([P, FIELD, NSLOT], i16, tag="idx")
                nc.sync.dma_start(out=idx_sb[:], in_=idx_d[:, :, :])

                # vg[p, f, t, :] = row[idx[t*128+p, f]]
                vg = pool.tile([P, FIELD, NTILES, RPAD], f32, tag="vg")
                nreg = nc.gpsimd.alloc_register()
                nc.gpsimd.reg_mov(nreg, BC)
                for f in range(FIELD):
                    nc.gpsimd.dma_gather(
                        out_ap=vg[:, f],
                        in_ap=taug_d[f * VOCAB:(f + 1) * VOCAB, :],
                        idxs_ap=idx_sb[:, f, :],
                        num_idxs=BC,
                        num_idxs_reg=nreg,
                        elem_size=RPAD,
                        single_packet=True,
                        queue_num=f % 4,
                    )

                # four partial reduces; the first three overlap later gathers
                GRP = [(0, 7), (7, 14), (14, 20), (20, 26)]
                acc4 = pool.tile([P, NTILES, 9, 4], f32, tag="acc4")
                for gi, (f0, f1) in enumerate(GRP):
                    nc.vector.tensor_reduce(
                        out=acc4[:, :, :, gi],
                        in_=vg[:, f0:f1, :, :9].rearrange(
                            "p f t c -> p t c f"
                        ),
                        axis=Ax.X,
                        op=Alu.add,
                    )
                acc = pool.tile([P, NTILES, 9], f32, tag="acc")
                nc.vector.tensor_reduce(
                    out=acc[:], in_=acc4[:], axis=Ax.X, op=Alu.add
                )
                # vsum was scaled by 1/sqrt(2) host-side, so
                # out = acc[8] + sum_k acc[k]^2 directly
                ssq = pool.tile([P, NTILES, K], f32, tag="ssq")
                nc.vector.tensor_tensor(
                    out=ssq[:], in0=acc[:, :, :K], in1=acc[:, :, :K],
                    op=Alu.mult,
                )
                s2 = pool.tile([P, NTILES], f32, tag="s2")
                nc.vector.tensor_reduce(
                    out=s2[:], in_=ssq[:], axis=Ax.X, op=Alu.add
                )
                out_all = pool.tile([P, NTILES], f32, tag="oa")
                nc.vector.tensor_tensor(
                    out=out_all[:], in0=s2[:], in1=acc[:, :, K],
                    op=Alu.add,
                )
                nc.sync.dma_start(
                    out=out_d[:, :].rearrange("(t p) one -> p (t one)", p=P),
                    in_=out_all[:],
                )
    nc.compile()
    return nc


def get_nc():
    if "nc" not in _cache:
        _cache["nc"] = _build_nc()
    return _cache["nc"]


def make_in_maps(inputs, offsets, w0, w, v):
    del offsets
    inp = np.asarray(inputs)
    idx16 = np.ascontiguousarray(
        inp.astype(np.int16).reshape(NCORES, BC, FIELD)
    )
    v_ = np.asarray(v, dtype=np.float32).reshape(TOTAL, FIELD, K)
    vsum = v_.sum(axis=1, dtype=np.float64)
    vsq = (vsum * vsum).sum(axis=1)
    w0_ = float(np.asarray(w0, np.float64).reshape(()))
    c = (np.asarray(w, np.float64).reshape(TOTAL) + w0_ / FIELD - 0.5 * vsq)
    taug = np.zeros((TOTAL, RPAD), dtype=np.float32)
    # scale vsum by 1/sqrt(2): sum_k (acc'_k)^2 == 0.5 * sum_k acc_k^2,
    # folding the 0.5 into the table and dropping a vector op on-device
    taug[:, :K] = (vsum / np.sqrt(2.0)).astype(np.float32)
    taug[:, K] = c.astype(np.float32)
    maps = []
    for i in range(NCORES):
        shard = idx16[i]                       # [BC, FIELD]
        wrapped = shard.reshape(NSLOT, 16, FIELD).transpose(1, 2, 0)
        rep = np.ascontiguousarray(np.tile(wrapped, (P // 16, 1, 1)))
        maps.append({"idx16": rep, "taug": taug})
    return maps


def kernel(inputs, offsets, w0, w, v):
    if _TRN_REPO not in sys.path:
        sys.path.insert(0, _TRN_REPO)
    from concourse.bass_utils import run_bass_kernel_spmd

    nc = get_nc()
    in_maps = make_in_maps(inputs, offsets, w0, w, v)
    res = run_bass_kernel_spmd(nc, in_maps, list(range(NCORES)))
    out = np.concatenate(
        [np.asarray(res.results[i]["out"]) for i in range(NCORES)], axis=0
    )
    return out.astype(np.float32)
